# revision 7
# baseline (speedup 1.0000x reference)
"""Causal self-attention (B=8, T=2048, C=128, H=4, D=32) on 8 trn2 NeuronCores.

Sharding: data-parallel over batch — core b handles batch element b.

Per-core algorithm (PE matmuls with fp16 moving operands = full rate at any N):
  xT = transpose(x)                      # PE transposes, [C, T] fp16
  qT, kT = (x @ Wq|k + b)^T              # w16 stationary, out [C,T] fp16
  v   = x @ Wv + bv                      # natural [T, C], packed into vaug
  vaug[tk-tile a] = [v_h | 1 | 0...]     # [128, 64] fp16 per head: the ones
                                         # column accumulates the denominator
  software pipeline over groups (tqbase, width, pair) of items (tk-tile a):
      S^T[tk,tq] = kT_h.T @ qT_h         # K=32 row-packed pairs, PSUM
      E = exp(S * 1/sqrt(32))            # ACT, fused scale, fp16 out
      (diag a: gpsimd affine_select zeroes the tk>tq triangle of E)
      psum_y += vaug_a.T @ E             # col-packed pairs, M=64; row 32 = sum E
  p_y columns finalize progressively (the diagonal PV of chunk m is the last
  writer of chunk m), so the close-out (reciprocal of the denominator rows,
  partition-broadcast via K=1 fp16 matmuls, multiply, then for pair-1 groups
  the projection chunk + output DMA) runs chunk-by-chunk through a background
  queue that trails the attention stream.  j=0 pair-0 is split into two
  256-wide warm-up groups so the first exp fires as soon as half of x-group-0
  has landed; (j=0, pair 1) runs last so the tail close-out is minimal.
"""

import sys

sys.path.insert(0, "/opt/trn_rl_repo")

import numpy as np

B, T, C = 8, 2048, 128
H, D = 4, 32
N_CORES = 8
TQ = 512
NT = T // 128     # 16 tk tiles
NJ = T // TQ      # 4 tq blocks
SCALE = 1.0 / np.sqrt(D)

_cache = {}


def _build():
    import concourse.bass as bass
    import concourse.mybir as mybir
    import concourse.tile as tile
    from concourse import bacc
    from concourse.masks import make_identity

    dt = mybir.dt
    AF = mybir.ActivationFunctionType
    nc = bacc.Bacc()

    x = nc.dram_tensor("x", [T, C], dt.float32, kind="ExternalInput")
    w_qkv = nc.dram_tensor("w_qkv", [C, 3 * C], dt.float32, kind="ExternalInput")
    b_qkv = nc.dram_tensor("b_qkv", [3 * C], dt.float32, kind="ExternalInput")
    w_proj = nc.dram_tensor("w_proj", [C, C], dt.float32, kind="ExternalInput")
    b_proj = nc.dram_tensor("b_proj", [C], dt.float32, kind="ExternalInput")
    y = nc.dram_tensor("y", [T, C], dt.float32, kind="ExternalOutput")

    with tile.TileContext(nc) as tc:
        with (
            nc.allow_low_precision(reason="fp16 matmuls; validated vs ref"),
            tc.tile_pool(name="const", bufs=1) as const,
            tc.tile_pool(name="big", bufs=1) as big,
            tc.tile_pool(name="sb", bufs=4) as sb,
            tc.tile_pool(name="esb", bufs=8) as esb,
            tc.tile_pool(name="ysb", bufs=2) as ysb,
            tc.tile_pool(name="ps_misc", bufs=2, space="PSUM") as ps_misc,
            tc.tile_pool(name="ps_s", bufs=2, space="PSUM") as ps_s,
            tc.tile_pool(name="ps_y", bufs=2, space="PSUM") as ps_y,
        ):
            # ---------------- t=0: act table load + input DMAs ----------------
            dumm = const.tile([1, 1], dt.float32)
            nc.vector.memset(dumm, 0.0)
            dummo = const.tile([1, 1], dt.float32)
            nc.scalar.activation(dummo, dumm, AF.Exp)

            # x half-group A on the gpsimd SWDGE queue (its own DGE path), w
            # first on the sync HWDGE queue: the first-exp critical path.
            xgA = const.tile([128, 2, C], dt.float32)
            nc.gpsimd.dma_start(
                out=xgA, in_=x[0:256, :].rearrange("(a p) c -> p a c", p=128)
            )
            w_sb = const.tile([128, 3 * C], dt.float32)
            nc.sync.dma_start(out=w_sb, in_=w_qkv[:, :])
            xgB = const.tile([128, 2, C], dt.float32)
            nc.sync.dma_start(
                out=xgB, in_=x[256:512, :].rearrange("(a p) c -> p a c", p=128)
            )
            bqk = const.tile([128, 2], dt.float32)
            nc.scalar.dma_start(
                out=bqk, in_=b_qkv[0:256].rearrange("(j p) -> p j", p=128)
            )
            xg = [None] * NJ
            for g in range(1, NJ):
                x_g = const.tile([128, 4, C], dt.float32, name=f"x_g{g}")
                xg[g] = x_g
                nc.sync.dma_start(
                    out=x_g,
                    in_=x[512 * g:512 * (g + 1), :].rearrange(
                        "(a p) c -> p a c", p=128),
                )
            # b_v / b_proj broadcast across partitions via stride-0 DMA reads
            bvb = const.tile([128, C], dt.float32)
            src = b_qkv[256:384]
            nc.scalar.dma_start(
                out=bvb,
                in_=bass.AP(tensor=src.tensor, offset=src.offset,
                            ap=[[0, 128], [1, C]]),
            )
            bpb = const.tile([128, C], dt.float32)
            src = b_proj[:]
            nc.scalar.dma_start(
                out=bpb,
                in_=bass.AP(tensor=src.tensor, offset=src.offset,
                            ap=[[0, 128], [1, C]]),
            )
            # w_proj pair tiles: rows 0-31/64-95 hold the pair's head rows,
            # the rest stays zero (junk lanes of the PV output contribute 0)
            wp_sb = const.tile([128, 2, C], dt.float32)
            for pair in range(2):
                nc.sync.dma_start(
                    out=wp_sb[0:32, pair, :],
                    in_=w_proj[64 * pair:64 * pair + 32, :],
                )
                nc.sync.dma_start(
                    out=wp_sb[64:96, pair, :],
                    in_=w_proj[64 * pair + 32:64 * pair + 64, :],
                )

            # identity for PE transposes (pool queue, after the xgA DMA)
            ident = const.tile([128, 128], dt.float32)
            make_identity(nc, ident)

            # p-state pre-warm: keep the PE busy with throwaway transposes
            # while the input DMAs are in flight so the real warm-up matmuls
            # run at full clock (the cost model ramps 0.65->2.4 GHz over 3us
            # of continuous execution)
            for _w in range(14):
                p_j = ps_s.tile([128, 1024], dt.float32, tag="s",
                                name=f"p_junk_{_w}")
                nc.tensor.transpose(p_j[:, 0:128], ident, ident)

            # fp16 qkv weights (stationary for qk, moving for v)
            w16 = const.tile([128, 3 * C], dt.float16)
            nc.vector.tensor_copy(w16, w_sb)

            # persistent activations
            xT = big.tile([128, T], dt.float16)        # [c, t]
            qkT = big.tile([128, 2, T], dt.float16)    # [c, {q,k}, t]
            vaug = big.tile([128, NT, 4, 64], dt.float16)

            # ---------------- warm-up: minimal path to the first exp --------
            def emit_warm_half(xh, a0):
                p_tr = ps_s.tile([128, 1024], dt.float32, tag="s",
                                 name=f"p_tr_{a0}")
                for k in range(2):
                    nc.tensor.transpose(p_tr[:, 128 * k:128 * (k + 1)],
                                        xh[:, k, :], ident)
                nc.vector.tensor_copy(
                    xT[:, 128 * a0:128 * a0 + 256], p_tr[:, 0:256])
                for ch in range(2):
                    p_qk = ps_misc.tile([128, 256], dt.float32, tag="misc",
                                        name=f"p_qk_w{a0}_{ch}")
                    nc.tensor.matmul(
                        p_qk,
                        w16[:, 128 * ch:128 * (ch + 1)],
                        xT[:, 128 * a0:128 * a0 + 256],
                        start=True, stop=True,
                    )
                    nc.vector.tensor_scalar_add(
                        qkT[:, ch, 128 * a0:128 * a0 + 256], p_qk,
                        bqk[:, ch:ch + 1],
                    )

            emit_warm_half(xgA, 0)

            # remaining constants / memsets (off the critical path)
            wp_pair = []
            for pair in range(2):
                wp_r = const.tile([128, C], dt.float16, name=f"wp_r_{pair}")
                nc.vector.memset(wp_r, 0.0)
                nc.vector.tensor_copy(wp_r[0:32, :], wp_sb[0:32, pair, :])
                nc.vector.tensor_copy(wp_r[64:96, :], wp_sb[64:96, pair, :])
                wp_pair.append(wp_r)
            onesf = const.tile([128, 64], dt.float16)
            nc.vector.memset(onesf, 1.0)
            nc.vector.memset(vaug[:, :, :, 32:64], 0.0)
            nc.vector.memset(vaug[:, :, :, 32:33], 1.0)

            def emit_x1(a):
                p_tr = ps_misc.tile([128, 128], dt.float32, tag="misc")
                nc.tensor.transpose(p_tr, xg[a // 4][:, a % 4, :], ident)
                nc.vector.tensor_copy(xT[:, 128 * a:128 * (a + 1)], p_tr)

            def emit_qk1(g, ch):
                p_qk = ps_misc.tile([128, TQ], dt.float32, tag="misc")
                nc.tensor.matmul(
                    p_qk,
                    w16[:, 128 * ch:128 * (ch + 1)],
                    xT[:, TQ * g:TQ * (g + 1)],
                    start=True, stop=True,
                )
                nc.vector.tensor_scalar_add(
                    qkT[:, ch, TQ * g:TQ * (g + 1)], p_qk, bqk[:, ch:ch + 1]
                )

            def emit_v1(a):
                p_v = ps_misc.tile([128, 128], dt.float32, tag="misc")
                nc.tensor.matmul(
                    p_v,
                    xT[:, 128 * a:128 * (a + 1)],
                    w16[:, 256:384],
                    start=True, stop=True,
                )
                nc.vector.tensor_add(
                    vaug[:, a, :, 0:32],
                    p_v.rearrange("p (h d) -> p h d", h=4),
                    bvb.rearrange("p (h d) -> p h d", h=4),
                )

            def emit_xqk(g):
                for a in range(4 * g, 4 * g + 4):
                    emit_x1(a)
                emit_qk1(g, 0)
                emit_qk1(g, 1)

            def emit_v(g):
                for a in range(4 * g, 4 * g + 4):
                    emit_v1(a)

            # ---------------- attention pipeline ----------------
            # group = (tqbase, width, pair); items a = 0..(tqbase+width)/128-1
            groups = [
                (0, 256, 0), (256, 256, 0),
                (512, 512, 0), (512, 512, 1),
                (1024, 512, 0), (1024, 512, 1),
                (1536, 512, 0), (1536, 512, 1),
                (0, 512, 1),
            ]
            items = []
            for gi, (tqbase, width, pair) in enumerate(groups):
                n_a = (tqbase + width) // 128
                for a in range(n_a):
                    items.append((gi, a))

            # ynorm pieces per (j, pair): list of (tqbase, width, tile)
            ypieces = {}
            p_ys = {}
            grp_state = {}   # gi -> dict(rrec, ynum, ynorm)

            from collections import deque
            bg = deque()

            def emit_pv(pend):
                p_yp, gi, e_p, a_p, off_p = pend
                tqbase, width, pair = groups[gi]
                n_a = (tqbase + width) // 128
                for ih in range(2):
                    nc.tensor.matmul(
                        p_yp[64 * ih:64 * (ih + 1), off_p:width],
                        vaug[:, a_p, 2 * pair + ih, :],
                        e_p[:, TQ * ih + off_p:TQ * ih + width],
                        start=(a_p == 0), stop=(a_p == n_a - 1),
                        tile_position=(0, 64 * ih),
                    )

            def emit_norm_a(gi, m):
                # reciprocal + numerator evac for chunk m (DVE)
                tqbase, width, pair = groups[gi]
                st = grp_state[gi]
                p_y = p_ys[gi]
                cm = slice(128 * m, 128 * (m + 1))
                nc.vector.reciprocal(st["rrec"][:, cm], p_y[:, cm])
                nc.vector.tensor_copy(st["ynum"][:, cm], p_y[:, cm])

            def emit_norm_b(gi, m):
                # partition-broadcast of the denominator rows (K=1 fp16
                # matmuls) + the normalize multiply
                tqbase, width, pair = groups[gi]
                st = grp_state[gi]
                cm = slice(128 * m, 128 * (m + 1))
                p_rb = ps_misc.tile([128, 128], dt.float32, tag="misc",
                                    name=f"p_rb_{gi}_{m}")
                for half in range(2):
                    nc.tensor.matmul(
                        p_rb[64 * half:64 * (half + 1), :],
                        onesf[32 + 64 * half:33 + 64 * half, :],
                        st["rrec"][32 + 64 * half:33 + 64 * half, cm],
                        start=True, stop=True,
                        tile_position=(32 + 64 * half, 64 * half),
                    )
                nc.vector.tensor_mul(st["ynorm"][:, cm], st["ynum"][:, cm],
                                     p_rb)

            def emit_proj(j, m):
                # output chunk t0 = 512j + 128m; stationary = the ynorm chunk
                # of each pair (fp16), moving = wp pair tiles (fp16)
                t0 = TQ * j + 128 * m
                p_o = ps_misc.tile([128, 128], dt.float32, tag="misc",
                                   name=f"p_o_{j}_{m}")
                for pr in range(2):
                    # pieces are stored with global tq coordinates
                    piece = None
                    for (pb, pw, ytile) in ypieces[(j, pr)]:
                        if pb <= t0 < pb + pw:
                            piece = ytile[:, t0 - pb:t0 - pb + 128]
                            break
                    nc.tensor.matmul(
                        p_o, piece, wp_pair[pr],
                        start=(pr == 0), stop=(pr == 1),
                    )
                o_t = sb.tile([128, 128], dt.float32, tag="out")
                nc.vector.tensor_add(o_t, p_o, bpb)
                nc.sync.dma_start(out=y[t0:t0 + 128, :], in_=o_t)

            def flush(pend):
                emit_pv(pend)
                _, gi, _, a_p, _ = pend
                tqbase, width, pair = groups[gi]
                r = a_p - tqbase // 128
                if r >= 0:
                    # chunk r of p_y is now final: close it out through bg
                    j = (tqbase + 128 * r) // TQ
                    jm = (tqbase + 128 * r - TQ * j) // 128
                    bg.append((0, lambda gi=gi, m=r: emit_norm_a(gi, m)))
                    bg.append((1, lambda gi=gi, m=r: emit_norm_b(gi, m)))
                    if pair == 1:
                        bg.append((2, lambda j=j, m=jm: emit_proj(j, m)))
                if bg:
                    bg.popleft()[1]()

            # deferred QKV emission points: (gi, a) -> thunks.  vaug tile a
            # must be written before any PV reading it flushes (LA=3 gives
            # three items of slack)
            defer_after = {
                (1, 0): [lambda: emit_v1(0), lambda: emit_v1(1)],
                (1, 1): [lambda: emit_xqk(1)],
                (1, 2): [lambda: emit_v1(2), lambda: emit_v1(3)],
                (1, 3): [lambda: emit_v(1)],
                (3, 1): [lambda: emit_xqk(2)],
                (3, 5): [lambda: emit_v(2)],
                (5, 1): [lambda: emit_xqk(3)],
                (5, 5): [lambda: emit_v(3)],
            }

            pends = []
            last_gi = len(groups) - 1
            for i_it, (gi, a) in enumerate(items):
                tqbase, width, pair = groups[gi]
                if a == 0:
                    p_y = ps_y.tile([128, width], dt.float32, tag="py",
                                    name=f"p_y_{gi}", padded_shape=[128, TQ])
                    p_ys[gi] = p_y
                    j = tqbase // TQ
                    rrec = ysb.tile([128, width], dt.float16, tag="rrec",
                                    name=f"rrec_{gi}", padded_shape=[128, TQ])
                    ynum = ysb.tile([128, width], dt.float32, tag="ynum",
                                    name=f"ynum_{gi}", padded_shape=[128, TQ])
                    ynorm = ysb.tile([128, width], dt.float16, tag="ynorm",
                                     name=f"ynorm_{gi}", bufs=9,
                                     padded_shape=[128, TQ])
                    grp_state[gi] = {"rrec": rrec, "ynum": ynum,
                                     "ynorm": ynorm}
                    ypieces.setdefault((j, pair), []).append(
                        (tqbase, width, ynorm))
                p_y = p_ys[gi]
                r = a - tqbase // 128
                off = 128 * r if r > 0 else 0
                p_s = ps_s.tile([128, 1024], dt.float32, tag="s")
                for ih, h in enumerate((2 * pair, 2 * pair + 1)):
                    nc.tensor.matmul(
                        p_s[:, TQ * ih + off:TQ * ih + width],
                        qkT[32 * h:32 * (h + 1), 1, 128 * a:128 * (a + 1)],
                        qkT[32 * h:32 * (h + 1), 0,
                            tqbase + off:tqbase + width],
                        start=True, stop=True,
                        tile_position=(32 * h, 0),
                    )
                e_t = esb.tile([128, 1024], dt.float16, tag="e")
                nc.scalar.activation(
                    e_t.rearrange("p (i f) -> p i f", i=2)[:, :, off:width],
                    p_s.rearrange("p (i f) -> p i f", i=2)[:, :, off:width],
                    AF.Exp,
                    scale=float(SCALE),
                )
                if r >= 0:
                    # diag tile: zero the causal triangle (tk > tq) on GPSIMD
                    sel = e_t.rearrange(
                        "p (i f) -> p i f", i=2)[:, :, off:off + 128]
                    nc.gpsimd.affine_select(
                        out=sel,
                        in_=sel,
                        compare_op=mybir.AluOpType.is_ge,
                        fill=0.0,
                        base=0,
                        pattern=[[0, 2], [1, 128]],
                        channel_multiplier=-1,
                    )
                pends.append((p_y, gi, e_t, a, off))
                la = 1 if gi == last_gi else 3
                while len(pends) > la:
                    flush(pends.pop(0))
                if i_it == 1:
                    # emit the B warm-up half after the 0A items so its chain
                    # overlaps the first exps
                    emit_warm_half(xgB, 2)
                for thunk in defer_after.get((gi, a), ()):
                    thunk()
            for pp in pends:
                flush(pp)
            # drain the close-out queue sorted by stage so same-stage DVE ops
            # run back-to-back instead of head-of-line blocking on the
            # cross-engine chain of a single chunk
            for _, thunk in sorted(bg, key=lambda kt: kt[0]):
                thunk()
            bg.clear()

    nc.compile()
    return nc


def _get_nc():
    if "nc" not in _cache:
        _cache["nc"] = _build()
    return _cache["nc"]


def run(inputs, trace=False):
    from concourse.bass_utils import run_bass_kernel_spmd

    nc = _get_nc()
    x = np.asarray(inputs["x"], dtype=np.float32)
    w_qkv = np.ascontiguousarray(np.asarray(inputs["w_qkv"], dtype=np.float32))
    b_qkv = np.ascontiguousarray(np.asarray(inputs["b_qkv"], dtype=np.float32))
    w_proj = np.ascontiguousarray(np.asarray(inputs["w_proj"], dtype=np.float32))
    b_proj = np.ascontiguousarray(np.asarray(inputs["b_proj"], dtype=np.float32))
    in_maps = [
        {
            "x": np.ascontiguousarray(x[b]),
            "w_qkv": w_qkv,
            "b_qkv": b_qkv,
            "w_proj": w_proj,
            "b_proj": b_proj,
        }
        for b in range(N_CORES)
    ]
    res = run_bass_kernel_spmd(
        nc, in_maps, core_ids=list(range(N_CORES)), trace=trace
    )
    out = np.stack([res.results[b]["y"] for b in range(N_CORES)], axis=0)
    return out, res


def kernel(**inputs) -> np.ndarray:
    out, _ = run(inputs, trace=False)
    return out


# revision 12
# speedup vs baseline: 1.0129x; 1.0129x over previous
"""Causal self-attention (B=8, T=2048, C=128, H=4, D=32) on 8 trn2 NeuronCores.

Sharding: data-parallel over batch — core b handles batch element b.

Per-core algorithm (PE matmuls with fp16 moving operands = full rate at any N):
  xT = transpose(x)                      # PE transposes, [C, T] fp16
  qT, kT = (x @ Wq|k + b)^T              # w16 stationary, out [C,T] fp16
  v   = x @ Wv + bv                      # natural [T, C], packed into vaug
  vaug[tk-tile a] = [v_h | 1 | 0...]     # [128, 64] fp16 per head: the ones
                                         # column accumulates the denominator
  software pipeline over groups (tqbase, width, pair) of items (tk-tile a):
      S^T[tk,tq] = kT_h.T @ qT_h         # K=32 row-packed pairs, PSUM
      E = exp(S * 1/sqrt(32))            # ACT, fused scale, fp16 out
      (diag a: gpsimd affine_select zeroes the tk>tq triangle of E)
      psum_y += vaug_a.T @ E             # col-packed pairs, M=64; row 32 = sum E
  p_y columns finalize progressively (the diagonal PV of chunk m is the last
  writer of chunk m), so the close-out (reciprocal of the denominator rows,
  partition-broadcast via K=1 fp16 matmuls, multiply, then for pair-1 groups
  the projection chunk + output DMA) runs chunk-by-chunk through a background
  queue that trails the attention stream.  j=0 pair-0 is split into two
  256-wide warm-up groups so the first exp fires as soon as half of x-group-0
  has landed; (j=0, pair 1) runs last so the tail close-out is minimal.
"""

import sys

sys.path.insert(0, "/opt/trn_rl_repo")

import numpy as np

B, T, C = 8, 2048, 128
H, D = 4, 32
N_CORES = 8
TQ = 512
NT = T // 128     # 16 tk tiles
NJ = T // TQ      # 4 tq blocks
SCALE = 1.0 / np.sqrt(D)

_cache = {}


def _build():
    import concourse.bass as bass
    import concourse.mybir as mybir
    import concourse.tile as tile
    from concourse import bacc
    from concourse.masks import make_identity

    dt = mybir.dt
    AF = mybir.ActivationFunctionType
    nc = bacc.Bacc()

    x = nc.dram_tensor("x", [T, C], dt.float32, kind="ExternalInput")
    w_qkv = nc.dram_tensor("w_qkv", [C, 3 * C], dt.float32, kind="ExternalInput")
    b_qkv = nc.dram_tensor("b_qkv", [3 * C], dt.float32, kind="ExternalInput")
    w_proj = nc.dram_tensor("w_proj", [C, C], dt.float32, kind="ExternalInput")
    b_proj = nc.dram_tensor("b_proj", [C], dt.float32, kind="ExternalInput")
    y = nc.dram_tensor("y", [T, C], dt.float32, kind="ExternalOutput")

    with tile.TileContext(nc) as tc:
        with (
            nc.allow_low_precision(reason="fp16 matmuls; validated vs ref"),
            tc.tile_pool(name="const", bufs=1) as const,
            tc.tile_pool(name="big", bufs=1) as big,
            tc.tile_pool(name="sb", bufs=4) as sb,
            tc.tile_pool(name="esb", bufs=8) as esb,
            tc.tile_pool(name="ysb", bufs=2) as ysb,
            tc.tile_pool(name="ps_misc", bufs=2, space="PSUM") as ps_misc,
            tc.tile_pool(name="ps_s", bufs=2, space="PSUM") as ps_s,
            tc.tile_pool(name="ps_y", bufs=2, space="PSUM") as ps_y,
        ):
            # ---------------- t=0: act table load + input DMAs ----------------
            dumm = const.tile([1, 1], dt.float32)
            nc.vector.memset(dumm, 0.0)
            dummo = const.tile([1, 1], dt.float32)
            nc.scalar.activation(dummo, dumm, AF.Exp)

            # first-exp critical path: x half-group A first on HWDGE, then w
            xgA = const.tile([128, 2, C], dt.float32)
            nc.sync.dma_start(
                out=xgA, in_=x[0:256, :].rearrange("(a p) c -> p a c", p=128)
            )
            w_sb = const.tile([128, 3 * C], dt.float32)
            nc.sync.dma_start(out=w_sb, in_=w_qkv[:, :])
            xgB = const.tile([128, 2, C], dt.float32)
            nc.sync.dma_start(
                out=xgB, in_=x[256:512, :].rearrange("(a p) c -> p a c", p=128)
            )
            bqk = const.tile([128, 2], dt.float32)
            nc.scalar.dma_start(
                out=bqk, in_=b_qkv[0:256].rearrange("(j p) -> p j", p=128)
            )
            xg = [None] * NJ
            for g in range(1, NJ):
                x_g = const.tile([128, 4, C], dt.float32, name=f"x_g{g}")
                xg[g] = x_g
                nc.sync.dma_start(
                    out=x_g,
                    in_=x[512 * g:512 * (g + 1), :].rearrange(
                        "(a p) c -> p a c", p=128),
                )
            # b_v / b_proj broadcast across partitions via stride-0 DMA reads
            bvb = const.tile([128, C], dt.float32)
            src = b_qkv[256:384]
            nc.scalar.dma_start(
                out=bvb,
                in_=bass.AP(tensor=src.tensor, offset=src.offset,
                            ap=[[0, 128], [1, C]]),
            )
            bpb = const.tile([128, C], dt.float32)
            src = b_proj[:]
            nc.scalar.dma_start(
                out=bpb,
                in_=bass.AP(tensor=src.tensor, offset=src.offset,
                            ap=[[0, 128], [1, C]]),
            )
            # w_proj pair tiles: rows 0-31/64-95 hold the pair's head rows,
            # the rest stays zero (junk lanes of the PV output contribute 0)
            wp_sb = const.tile([128, 2, C], dt.float32)
            for pair in range(2):
                nc.sync.dma_start(
                    out=wp_sb[0:32, pair, :],
                    in_=w_proj[64 * pair:64 * pair + 32, :],
                )
                nc.sync.dma_start(
                    out=wp_sb[64:96, pair, :],
                    in_=w_proj[64 * pair + 32:64 * pair + 64, :],
                )

            # identity for PE transposes (pool queue, after the xgA DMA)
            ident = const.tile([128, 128], dt.float32)
            make_identity(nc, ident)

            # p-state pre-warm: keep the PE busy with throwaway transposes
            # while the input DMAs are in flight so the real warm-up matmuls
            # run at full clock (the cost model ramps 0.65->2.4 GHz over 3us
            # of continuous execution)
            for _w in range(9):
                p_j = ps_s.tile([128, 1024], dt.float32, tag="s",
                                name=f"p_junk_{_w}")
                nc.tensor.transpose(p_j[:, 0:128], ident, ident)

            # fp16 qkv weights (stationary for qk, moving for v)
            w16 = const.tile([128, 3 * C], dt.float16)
            nc.vector.tensor_copy(w16, w_sb)

            # persistent activations
            xT = big.tile([128, T], dt.float16)        # [c, t]
            qkT = big.tile([128, 2, T], dt.float16)    # [c, {q,k}, t]
            vaug = big.tile([128, NT, 4, 64], dt.float16)

            # ---------------- warm-up: minimal path to the first exp --------
            def emit_warm_half(xh, a0):
                p_tr = ps_s.tile([128, 1024], dt.float32, tag="s",
                                 name=f"p_tr_{a0}")
                for k in range(2):
                    nc.tensor.transpose(p_tr[:, 128 * k:128 * (k + 1)],
                                        xh[:, k, :], ident)
                nc.vector.tensor_copy(
                    xT[:, 128 * a0:128 * a0 + 256], p_tr[:, 0:256])
                for ch in range(2):
                    p_qk = ps_misc.tile([128, 256], dt.float32, tag="misc",
                                        name=f"p_qk_w{a0}_{ch}")
                    nc.tensor.matmul(
                        p_qk,
                        w16[:, 128 * ch:128 * (ch + 1)],
                        xT[:, 128 * a0:128 * a0 + 256],
                        start=True, stop=True,
                    )
                    nc.vector.tensor_scalar_add(
                        qkT[:, ch, 128 * a0:128 * a0 + 256], p_qk,
                        bqk[:, ch:ch + 1],
                    )

            emit_warm_half(xgA, 0)

            # remaining constants / memsets (off the critical path).  The
            # wp_r copies wait on the late wp DMAs, so they are deferred into
            # the pipeline (emitting them here would head-of-line block the
            # in-order DVE queue while the DMA is in flight).
            wp_pair = []
            for pair in range(2):
                wp_r = const.tile([128, C], dt.float16, name=f"wp_r_{pair}")
                nc.vector.memset(wp_r, 0.0)
                wp_pair.append(wp_r)
            onesf = const.tile([128, 64], dt.float16)
            nc.vector.memset(onesf, 1.0)
            nc.vector.memset(vaug[:, :, :, 32:64], 0.0)
            nc.vector.memset(vaug[:, :, :, 32:33], 1.0)

            def emit_wp():
                for pair in range(2):
                    nc.vector.tensor_copy(
                        wp_pair[pair][0:32, :], wp_sb[0:32, pair, :])
                    nc.vector.tensor_copy(
                        wp_pair[pair][64:96, :], wp_sb[64:96, pair, :])

            def emit_x1(a):
                p_tr = ps_misc.tile([128, 128], dt.float32, tag="misc")
                nc.tensor.transpose(p_tr, xg[a // 4][:, a % 4, :], ident)
                nc.vector.tensor_copy(xT[:, 128 * a:128 * (a + 1)], p_tr)

            def emit_qk1(g, ch):
                p_qk = ps_misc.tile([128, TQ], dt.float32, tag="misc")
                nc.tensor.matmul(
                    p_qk,
                    w16[:, 128 * ch:128 * (ch + 1)],
                    xT[:, TQ * g:TQ * (g + 1)],
                    start=True, stop=True,
                )
                nc.vector.tensor_scalar_add(
                    qkT[:, ch, TQ * g:TQ * (g + 1)], p_qk, bqk[:, ch:ch + 1]
                )

            def emit_v1(a):
                p_v = ps_misc.tile([128, 128], dt.float32, tag="misc")
                nc.tensor.matmul(
                    p_v,
                    xT[:, 128 * a:128 * (a + 1)],
                    w16[:, 256:384],
                    start=True, stop=True,
                )
                nc.vector.tensor_add(
                    vaug[:, a, :, 0:32],
                    p_v.rearrange("p (h d) -> p h d", h=4),
                    bvb.rearrange("p (h d) -> p h d", h=4),
                )

            def emit_xqk(g):
                for a in range(4 * g, 4 * g + 4):
                    emit_x1(a)
                emit_qk1(g, 0)
                emit_qk1(g, 1)

            def emit_v(g):
                for a in range(4 * g, 4 * g + 4):
                    emit_v1(a)

            # ---------------- attention pipeline ----------------
            # group = (tqbase, width, pair); items a = 0..(tqbase+width)/128-1
            groups = [
                (0, 256, 0), (256, 256, 0),
                (512, 512, 0), (512, 512, 1),
                (1024, 512, 0), (1024, 512, 1),
                (1536, 512, 0), (1536, 512, 1),
                (0, 512, 1),
            ]
            items = []
            for gi, (tqbase, width, pair) in enumerate(groups):
                n_a = (tqbase + width) // 128
                for a in range(n_a):
                    items.append((gi, a))

            # ynorm pieces per (j, pair): list of (tqbase, width, tile)
            ypieces = {}
            p_ys = {}
            grp_state = {}   # gi -> dict(rrec, ynum, ynorm)

            from collections import deque
            bg = deque()

            def emit_pv(pend):
                p_yp, gi, e_p, a_p, off_p = pend
                tqbase, width, pair = groups[gi]
                n_a = (tqbase + width) // 128
                for ih in range(2):
                    nc.tensor.matmul(
                        p_yp[64 * ih:64 * (ih + 1), off_p:width],
                        vaug[:, a_p, 2 * pair + ih, :],
                        e_p[:, TQ * ih + off_p:TQ * ih + width],
                        start=(a_p == 0), stop=(a_p == n_a - 1),
                        tile_position=(0, 64 * ih),
                    )

            def emit_norm_a(gi, m):
                # reciprocal + numerator evac for chunk m (DVE)
                tqbase, width, pair = groups[gi]
                st = grp_state[gi]
                p_y = p_ys[gi]
                cm = slice(128 * m, 128 * (m + 1))
                nc.vector.reciprocal(st["rrec"][:, cm], p_y[:, cm])
                nc.vector.tensor_copy(st["ynum"][:, cm], p_y[:, cm])

            def emit_norm_b(gi, m):
                # partition-broadcast of the denominator rows (K=1 fp16
                # matmuls) + the normalize multiply
                tqbase, width, pair = groups[gi]
                st = grp_state[gi]
                cm = slice(128 * m, 128 * (m + 1))
                p_rb = ps_misc.tile([128, 128], dt.float32, tag="misc",
                                    name=f"p_rb_{gi}_{m}")
                for half in range(2):
                    nc.tensor.matmul(
                        p_rb[64 * half:64 * (half + 1), :],
                        onesf[32 + 64 * half:33 + 64 * half, :],
                        st["rrec"][32 + 64 * half:33 + 64 * half, cm],
                        start=True, stop=True,
                        tile_position=(32 + 64 * half, 64 * half),
                    )
                nc.vector.tensor_mul(st["ynorm"][:, cm], st["ynum"][:, cm],
                                     p_rb)

            def emit_proj(j, m):
                # output chunk t0 = 512j + 128m; stationary = the ynorm chunk
                # of each pair (fp16), moving = wp pair tiles (fp16)
                t0 = TQ * j + 128 * m
                p_o = ps_misc.tile([128, 128], dt.float32, tag="misc",
                                   name=f"p_o_{j}_{m}")
                for pr in range(2):
                    # pieces are stored with global tq coordinates
                    piece = None
                    for (pb, pw, ytile) in ypieces[(j, pr)]:
                        if pb <= t0 < pb + pw:
                            piece = ytile[:, t0 - pb:t0 - pb + 128]
                            break
                    nc.tensor.matmul(
                        p_o, piece, wp_pair[pr],
                        start=(pr == 0), stop=(pr == 1),
                    )
                o_t = sb.tile([128, 128], dt.float32, tag="out")
                nc.vector.tensor_add(o_t, p_o, bpb)
                nc.sync.dma_start(out=y[t0:t0 + 128, :], in_=o_t)

            def flush(pend):
                emit_pv(pend)
                _, gi, _, a_p, _ = pend
                tqbase, width, pair = groups[gi]
                r = a_p - tqbase // 128
                if r >= 0:
                    # chunk r of p_y is now final.  The reciprocal/numerator
                    # stage is DVE-only, so it can't head-of-line block the
                    # PE stream: emit inline.  The stages containing PE work
                    # that waits on DVE results trail through bg.
                    j = (tqbase + 128 * r) // TQ
                    jm = (tqbase + 128 * r - TQ * j) // 128
                    emit_norm_a(gi, r)
                    bg.append((1, lambda gi=gi, m=r: emit_norm_b(gi, m)))
                    if pair == 1:
                        bg.append((2, lambda j=j, m=jm: emit_proj(j, m)))
                for _ in range(2):
                    if bg:
                        bg.popleft()[1]()

            # deferred QKV emission points: (gi, a) -> thunks.  vaug tile a
            # must be written before any PV reading it flushes (LA=3 gives
            # three items of slack)
            defer_after = {
                (1, 0): [lambda: emit_v1(0), lambda: emit_v1(1)],
                (1, 1): [lambda: emit_xqk(1)],
                (1, 2): [lambda: emit_v1(2), lambda: emit_v1(3)],
                (1, 3): [lambda: emit_v(1)],
                (2, 5): [emit_wp],
                (3, 1): [lambda: emit_xqk(2)],
                (3, 5): [lambda: emit_v(2)],
                (5, 1): [lambda: emit_xqk(3)],
                (5, 5): [lambda: emit_v(3)],
            }

            pends = []
            last_gi = len(groups) - 1
            for i_it, (gi, a) in enumerate(items):
                tqbase, width, pair = groups[gi]
                if a == 0:
                    p_y = ps_y.tile([128, width], dt.float32, tag="py",
                                    name=f"p_y_{gi}", padded_shape=[128, TQ])
                    p_ys[gi] = p_y
                    j = tqbase // TQ
                    rrec = ysb.tile([128, width], dt.float16, tag="rrec",
                                    name=f"rrec_{gi}", padded_shape=[128, TQ])
                    ynum = ysb.tile([128, width], dt.float32, tag="ynum",
                                    name=f"ynum_{gi}", padded_shape=[128, TQ])
                    ynorm = ysb.tile([128, width], dt.float16, tag="ynorm",
                                     name=f"ynorm_{gi}", bufs=9,
                                     padded_shape=[128, TQ])
                    grp_state[gi] = {"rrec": rrec, "ynum": ynum,
                                     "ynorm": ynorm}
                    ypieces.setdefault((j, pair), []).append(
                        (tqbase, width, ynorm))
                p_y = p_ys[gi]
                r = a - tqbase // 128
                off = 128 * r if r > 0 else 0
                p_s = ps_s.tile([128, 1024], dt.float32, tag="s")
                for ih, h in enumerate((2 * pair, 2 * pair + 1)):
                    nc.tensor.matmul(
                        p_s[:, TQ * ih + off:TQ * ih + width],
                        qkT[32 * h:32 * (h + 1), 1, 128 * a:128 * (a + 1)],
                        qkT[32 * h:32 * (h + 1), 0,
                            tqbase + off:tqbase + width],
                        start=True, stop=True,
                        tile_position=(32 * h, 0),
                    )
                e_t = esb.tile([128, 1024], dt.float16, tag="e")
                nc.scalar.activation(
                    e_t.rearrange("p (i f) -> p i f", i=2)[:, :, off:width],
                    p_s.rearrange("p (i f) -> p i f", i=2)[:, :, off:width],
                    AF.Exp,
                    scale=float(SCALE),
                )
                if r >= 0:
                    # diag tile: zero the causal triangle (tk > tq) on GPSIMD
                    sel = e_t.rearrange(
                        "p (i f) -> p i f", i=2)[:, :, off:off + 128]
                    nc.gpsimd.affine_select(
                        out=sel,
                        in_=sel,
                        compare_op=mybir.AluOpType.is_ge,
                        fill=0.0,
                        base=0,
                        pattern=[[0, 2], [1, 128]],
                        channel_multiplier=-1,
                    )
                pends.append((p_y, gi, e_t, a, off))
                la = 1 if gi == last_gi else 3
                while len(pends) > la:
                    flush(pends.pop(0))
                if i_it == 1:
                    # emit the B warm-up half after the 0A items so its chain
                    # overlaps the first exps
                    emit_warm_half(xgB, 2)
                for thunk in defer_after.get((gi, a), ()):
                    thunk()
            for pp in pends:
                flush(pp)
            # drain the close-out queue sorted by stage so same-stage DVE ops
            # run back-to-back instead of head-of-line blocking on the
            # cross-engine chain of a single chunk
            for _, thunk in sorted(bg, key=lambda kt: kt[0]):
                thunk()
            bg.clear()

    nc.compile()
    return nc


def _get_nc():
    if "nc" not in _cache:
        _cache["nc"] = _build()
    return _cache["nc"]


def run(inputs, trace=False):
    from concourse.bass_utils import run_bass_kernel_spmd

    nc = _get_nc()
    x = np.asarray(inputs["x"], dtype=np.float32)
    w_qkv = np.ascontiguousarray(np.asarray(inputs["w_qkv"], dtype=np.float32))
    b_qkv = np.ascontiguousarray(np.asarray(inputs["b_qkv"], dtype=np.float32))
    w_proj = np.ascontiguousarray(np.asarray(inputs["w_proj"], dtype=np.float32))
    b_proj = np.ascontiguousarray(np.asarray(inputs["b_proj"], dtype=np.float32))
    in_maps = [
        {
            "x": np.ascontiguousarray(x[b]),
            "w_qkv": w_qkv,
            "b_qkv": b_qkv,
            "w_proj": w_proj,
            "b_proj": b_proj,
        }
        for b in range(N_CORES)
    ]
    res = run_bass_kernel_spmd(
        nc, in_maps, core_ids=list(range(N_CORES)), trace=trace
    )
    out = np.stack([res.results[b]["y"] for b in range(N_CORES)], axis=0)
    return out, res


def kernel(**inputs) -> np.ndarray:
    out, _ = run(inputs, trace=False)
    return out
